# revision 1
# baseline (speedup 1.0000x reference)
"""Trainium2 Bass kernel for the 21x21 correlation (cost volume) module.

Math: out[b, di*21+dj, i, j] = sum_c x1p[b, c, i+di, j+dj] * x2[b, c, i, j]
where x1p is x1 zero-padded by 10 on both spatial dims, di,dj in [0,21).

Strategy (8 NeuronCores, SPMD, no collectives):
  - Shard: batch (4) x W-halves (2). Core k -> (b = k//2, rows i in
    [64*(k%2), 64*(k%2)+64)). x1 shipped with a 10-row halo and +-10
    column padding, zero-filled on the host.
  - On-core: channels C=128 live on the SBUF partition dim (= matmul
    contraction K). For each 8x16 pixel patch, one 128-wide stationary
    operand (the x2 pixels) is multiplied against the streamed 28x36
    window of x1 positions (two matmuls of N=504), producing the
    all-pairs patch product PSUM[pixel, position]. That is copied to
    SBUF (ScalarE + VectorE in parallel) and DMA'd out as a dense
    [128, 1008] block per patch.
  - The band extraction (pixel-relative displacement gather) is a pure
    shear, which no uniform access pattern on the compute engines can
    express; it is done for free on the host with as_strided over the
    gathered [8, 8, 128, 1008] per-core output.
"""
import sys

if "/opt/trn_rl_repo" not in sys.path:
    sys.path.insert(0, "/opt/trn_rl_repo")

import numpy as np
from numpy.lib.stride_tricks import as_strided

import concourse.bass as bass
import concourse.mybir as mybir
import concourse.tile as tile
from concourse import bacc
from concourse.bass_utils import run_bass_kernel_spmd

B, C, W, H = 4, 128, 128, 128
DW = 21          # displacement window (per axis)
PAD = 10
N_CORES = 8
IB, JB = 8, 8            # patch grid per core (8 i-blocks x 8 j-blocks)
PI, PJ = 8, 16           # patch shape (pixels)
RW, QW = PI + DW - 1, PJ + DW - 1    # streamed window 28 x 36
NSTREAM = RW * QW        # 1008
HALO_ROWS = 64 + 2 * PAD     # 84
PADDED_COLS = H + 2 * PAD    # 148

# Matmul input dtype: float32r = full-rate reduced-precision fp32 matmul
# (fp32 storage). Flip to mybir.dt.float32 for exact-but-4x-slower.
MM_DT = mybir.dt.float32r

_CACHE = {}


def _build_program():
    nc = bacc.Bacc("TRN2", target_bir_lowering=False, debug=False,
                   num_devices=N_CORES)
    x1h = nc.dram_tensor("x1h", [C, HALO_ROWS, PADDED_COLS], mybir.dt.float32,
                         kind="ExternalInput")
    # x2 shipped patch-major: [c, ib, jb, pi*pj] so each patch's stationary
    # operand is a single contiguous 128-element free run.
    x2s = nc.dram_tensor("x2s", [C, IB, JB, PI * PJ], MM_DT,
                         kind="ExternalInput")
    outp = nc.dram_tensor("outp", [IB, JB, 128, NSTREAM], mybir.dt.float32,
                          kind="ExternalOutput")

    with tile.TileContext(nc) as tc:
        with (
            tc.tile_pool(name="singles", bufs=1) as singles,
            tc.tile_pool(name="outs", bufs=6) as outs,
            tc.tile_pool(name="repack", bufs=3) as repack,
            tc.tile_pool(name="psum", bufs=4, space="PSUM") as psum,
        ):
            x1_sb = singles.tile([C, HALO_ROWS, PADDED_COLS], mybir.dt.float32)
            x2_sb = singles.tile([C, IB, JB, PI * PJ], MM_DT)
            nc.sync.dma_start(out=x1_sb, in_=x1h[:, :, :])
            nc.sync.dma_start(out=x2_sb, in_=x2s[:, :, :, :])

            for ib in range(IB):
                for jb in range(JB):
                    lhsT = x2_sb[:, ib, jb, :]
                    ps0 = psum.tile([128, 504], mybir.dt.float32, name="ps0")
                    ps1 = psum.tile([128, 504], mybir.dt.float32, name="ps1")
                    # Repack the strided 28x36 x1 window into a contiguous
                    # run so the matmul rhs has a single free dim.
                    rp = repack.tile([128, RW, QW], MM_DT)
                    nc.scalar.copy(out=rp,
                                   in_=x1_sb[:, ib * PI:ib * PI + RW,
                                             jb * PJ:jb * PJ + QW])
                    rpf = rp.rearrange("p a b -> p (a b)")
                    nc.tensor.matmul(ps0, lhsT=lhsT,
                                     rhs=rpf[:, 0:504],
                                     start=True, stop=True)
                    nc.tensor.matmul(ps1, lhsT=lhsT,
                                     rhs=rpf[:, 504:NSTREAM],
                                     start=True, stop=True)
                    ot = outs.tile([128, NSTREAM], mybir.dt.float32)
                    nc.vector.tensor_copy(ot[:, 0:504], ps0)
                    nc.vector.tensor_copy(ot[:, 504:NSTREAM], ps1)
                    nc.sync.dma_start(out=outp[ib, jb], in_=ot)

    nc.finalize()
    return nc


def _shard_inputs(x1, x2):
    in_maps = []
    for k in range(N_CORES):
        b, half = divmod(k, 2)
        i0 = 64 * half
        x2sh = np.ascontiguousarray(
            x2[b][:, i0:i0 + 64, :]
            .reshape(C, IB, PI, JB, PJ)
            .transpose(0, 1, 3, 2, 4)
            .reshape(C, IB, JB, PI * PJ)
        )
        x1sh = np.zeros((C, HALO_ROWS, PADDED_COLS), np.float32)
        rlo, rhi = i0 - PAD, i0 + 64 + PAD
        slo, shi = max(rlo, 0), min(rhi, W)
        x1sh[:, slo - rlo:shi - rlo, PAD:PAD + H] = x1[b][:, slo:shi, :]
        in_maps.append({"x1h": x1sh, "x2s": x2sh})
    return in_maps


def _gather(results):
    out = np.empty((B, DW * DW, W, H), np.float32)
    for k in range(N_CORES):
        b, half = divmod(k, 2)
        i0 = 64 * half
        O = np.ascontiguousarray(results[k]["outp"])  # [8, 8, 128, 1008]
        e = O.itemsize
        s = O.strides
        sv = as_strided(
            O,
            shape=(IB, PI, JB, PJ, DW, DW),
            strides=(s[0], PJ * NSTREAM * e + QW * e, s[1],
                     NSTREAM * e + e, QW * e, e),
        )
        out[b, :, i0:i0 + 64, :] = (
            sv.transpose(4, 5, 0, 1, 2, 3).reshape(DW * DW, 64, H)
        )
    return out


def kernel(x1, x2):
    x1 = np.asarray(x1, dtype=np.float32)
    x2 = np.asarray(x2, dtype=np.float32)
    if "nc" not in _CACHE:
        _CACHE["nc"] = _build_program()
    nc = _CACHE["nc"]
    in_maps = _shard_inputs(x1, x2)
    res = run_bass_kernel_spmd(nc, in_maps, list(range(N_CORES)))
    return _gather(res.results)



# revision 3
# speedup vs baseline: 2.0924x; 2.0924x over previous
"""Trainium2 Bass kernel for the 21x21 correlation (cost volume) module.

Math: out[b, di*21+dj, i, j] = sum_c x1p[b, c, i+di, j+dj] * x2[b, c, i, j]
where x1p is x1 zero-padded by 10 on both spatial dims, di,dj in [0,21).

Strategy (8 NeuronCores, SPMD, no collectives):
  - Shard: batch (4) x W-halves (2). Core k -> (b = k//2, rows i in
    [64*(k%2), 64*(k%2)+64)). x1 shipped bf16 with a 10-row halo and +-10
    column padding (zero-filled on host); x2 shipped bf16 patch-major.
  - On-core: channels C=128 on the SBUF partition dim (matmul K). Pixels
    are processed in 16x8 patches (PI=16 rows x PJ=8 cols = 128 = M). Each
    patch needs the 36x28 window of padded x1. The matmul rhs must be a
    single contiguous free run, so each window ROW is its own matmul
    (N=28), 36 per patch, accumulated side by side into two 504-col PSUM
    banks. DVE and ACT evacuate one bank each, converting fp32->fp16 into
    a whole-row SBUF buffer [128, 16*1008].
  - Output trim: pixel row pi only needs window rows pi..pi+21 (588 of
    1008 values). One DMA per (ib, pixel-row-group g) reads partitions
    [8g, 8g+8) at free offset g*28 -- a partition-uniform (legal) AP --
    so only 588/1008 of the product ever reaches DRAM, as fp16.
  - Host: shear fp16 [.., 21, 28] windows by pj (as_strided) to extract
    the 21x21 displacement block per pixel; cast fp32.
"""
import sys

if "/opt/trn_rl_repo" not in sys.path:
    sys.path.insert(0, "/opt/trn_rl_repo")

import numpy as np
import ml_dtypes
from numpy.lib.stride_tricks import as_strided

import concourse.bass as bass
import concourse.mybir as mybir
import concourse.tile as tile
from concourse import bacc
from concourse.bass_utils import run_bass_kernel_spmd

B, C, W, H = 4, 128, 128, 128
DW = 21          # displacement window (per axis)
PAD = 10
N_CORES = 8
PI, PJ = 16, 8           # patch shape (pixel rows x pixel cols)
IB, JB = 64 // PI, H // PJ          # 4 x 16 patch grid per core
WR, WQ = PI + DW - 1, PJ + DW - 1   # window 36 x 28
NPP = WR * WQ            # 1008 window positions per patch
HNP = NPP // 2           # 504 = one PSUM bank worth
TRIM = DW * WQ           # 588 values kept per pixel (rows pi..pi+21)
F = JB * NPP             # 16128 row-buffer free size
HALO_ROWS = 64 + 2 * PAD     # 84
PADDED_COLS = H + 2 * PAD    # 148

BF16 = ml_dtypes.bfloat16

_CACHE = {}


def _build_program():
    nc = bacc.Bacc("TRN2", target_bir_lowering=False, debug=False,
                   num_devices=N_CORES)
    x1h = nc.dram_tensor("x1h", [C, HALO_ROWS, PADDED_COLS],
                         mybir.dt.bfloat16, kind="ExternalInput")
    x2s = nc.dram_tensor("x2s", [C, IB, JB, 128], mybir.dt.bfloat16,
                         kind="ExternalInput")
    outp = nc.dram_tensor("outp", [IB, PI, PJ, JB, TRIM], mybir.dt.float16,
                          kind="ExternalOutput")

    with tile.TileContext(nc) as tc:
        with (
            tc.tile_pool(name="singles", bufs=1) as singles,
            tc.tile_pool(name="rowbuf", bufs=2) as rowbuf,
            tc.tile_pool(name="psum", bufs=4, space="PSUM") as psum,
        ):
            x1_sb = singles.tile([C, HALO_ROWS, PADDED_COLS],
                                 mybir.dt.bfloat16)
            x2_sb = singles.tile([C, IB, JB, 128], mybir.dt.bfloat16)
            # chunked input loads so the first patches start early
            nc.sync.dma_start(out=x1_sb[:, 0:36, :], in_=x1h[:, 0:36, :])
            nc.sync.dma_start(out=x2_sb[:, 0:1], in_=x2s[:, 0:1])
            nc.sync.dma_start(out=x1_sb[:, 36:68, :], in_=x1h[:, 36:68, :])
            nc.sync.dma_start(out=x2_sb[:, 1:2], in_=x2s[:, 1:2])
            nc.sync.dma_start(out=x1_sb[:, 68:HALO_ROWS, :],
                              in_=x1h[:, 68:HALO_ROWS, :])
            nc.sync.dma_start(out=x2_sb[:, 2:4], in_=x2s[:, 2:4])

            for ib in range(IB):
                rb = rowbuf.tile([128, F], mybir.dt.float16)
                for jb in range(JB):
                    lhsT = x2_sb[:, ib, jb, :]
                    ps0 = psum.tile([128, HNP], mybir.dt.float32, name="ps0")
                    ps1 = psum.tile([128, HNP], mybir.dt.float32, name="ps1")
                    r0, c0 = ib * PI, jb * PJ
                    for r in range(WR // 2):
                        nc.tensor.matmul(
                            ps0[:, r * WQ:(r + 1) * WQ], lhsT=lhsT,
                            rhs=x1_sb[:, r0 + r, c0:c0 + WQ],
                            start=True, stop=True)
                    for r in range(WR // 2):
                        nc.tensor.matmul(
                            ps1[:, r * WQ:(r + 1) * WQ], lhsT=lhsT,
                            rhs=x1_sb[:, r0 + WR // 2 + r, c0:c0 + WQ],
                            start=True, stop=True)
                    base = jb * NPP
                    nc.vector.tensor_copy(rb[:, base:base + HNP], ps0)
                    nc.scalar.copy(out=rb[:, base + HNP:base + NPP], in_=ps1)
                # trimmed output: group g = pixel row pi, partitions
                # [8g, 8g+8), free offset g*28, runs of 588
                for g in range(PI):
                    src = bass.AP(rb.tensor, PJ * g * F + g * WQ,
                                  [[F, PJ], [NPP, JB], [1, TRIM]])
                    eng = nc.gpsimd if g % 4 == 3 else nc.sync
                    eng.dma_start(out=outp[ib, g], in_=src)

    nc.finalize()
    return nc


def _shard_inputs(x1, x2):
    x1 = np.asarray(x1, dtype=np.float32)
    x2 = np.asarray(x2, dtype=np.float32)
    x1b = x1.astype(BF16)
    x2b = x2.astype(BF16)
    in_maps = []
    for k in range(N_CORES):
        b, half = divmod(k, 2)
        i0 = 64 * half
        x2sh = np.ascontiguousarray(
            x2b[b][:, i0:i0 + 64, :]
            .reshape(C, IB, PI, JB, PJ)
            .transpose(0, 1, 3, 2, 4)
            .reshape(C, IB, JB, 128)
        )
        x1sh = np.zeros((C, HALO_ROWS, PADDED_COLS), BF16)
        rlo, rhi = i0 - PAD, i0 + 64 + PAD
        slo, shi = max(rlo, 0), min(rhi, W)
        x1sh[:, slo - rlo:shi - rlo, PAD:PAD + H] = x1b[b][:, slo:shi, :]
        in_maps.append({"x1h": x1sh, "x2s": x2sh})
    return in_maps


def _gather(results):
    out = np.empty((B, DW * DW, W, H), np.float32)
    for k in range(N_CORES):
        b, half = divmod(k, 2)
        i0 = 64 * half
        O = np.ascontiguousarray(results[k]["outp"])  # [4, 16, 8, 16, 588]
        O6 = O.reshape(IB, PI, PJ, JB, DW, WQ)
        s = O6.strides
        # pixel (pi, pj) row di holds window cols [0, 28); its dj block
        # starts at col pj: bump the pj stride by the dj stride
        V = as_strided(O6, shape=(IB, PI, PJ, JB, DW, DW),
                       strides=(s[0], s[1], s[2] + s[5], s[3], s[4], s[5]))
        arr = V.transpose(4, 5, 0, 1, 3, 2).reshape(DW * DW, 64, H)
        out[b, :, i0:i0 + 64, :] = arr
    return out


def kernel(x1, x2):
    if "nc" not in _CACHE:
        _CACHE["nc"] = _build_program()
    nc = _CACHE["nc"]
    in_maps = _shard_inputs(x1, x2)
    res = run_bass_kernel_spmd(nc, in_maps, list(range(N_CORES)))
    return _gather(res.results)


# revision 5
# speedup vs baseline: 2.1620x; 1.0333x over previous
"""Trainium2 Bass kernel for the 21x21 correlation (cost volume) module.

Math: out[b, di*21+dj, i, j] = sum_c x1p[b, c, i+di, j+dj] * x2[b, c, i, j]
where x1p is x1 zero-padded by 10 on both spatial dims, di,dj in [0,21).

Strategy (8 NeuronCores, SPMD, no collectives):
  - Shard: batch (4) x W-halves (2). Core k -> (b = k//2, rows i in
    [64*(k%2), 64*(k%2)+64)). x1 shipped bf16 with a 10-row halo and +-10
    column padding (zero-filled on host); x2 shipped bf16 patch-major.
  - On-core: channels C=128 on the SBUF partition dim (matmul K). Pixels
    are processed in 16x8 patches (PI=16 rows x PJ=8 cols = 128 = M). Each
    patch needs the 36x28 window of padded x1. The matmul rhs must be a
    single contiguous free run, so each window ROW is its own matmul
    (N=28), 36 per patch, accumulated side by side into two 504-col PSUM
    banks. DVE and ACT evacuate one bank each, converting fp32->fp16 into
    a whole-row SBUF buffer [128, 16*1008].
  - Output trim: pixel row pi only needs window rows pi..pi+21 (588 of
    1008 values). One DMA per (ib, pixel-row-group g) reads partitions
    [8g, 8g+8) at free offset g*28 -- a partition-uniform (legal) AP --
    so only 588/1008 of the product ever reaches DRAM, as fp16.
  - Host: shear fp16 [.., 21, 28] windows by pj (as_strided) to extract
    the 21x21 displacement block per pixel; cast fp32.
"""
import sys

if "/opt/trn_rl_repo" not in sys.path:
    sys.path.insert(0, "/opt/trn_rl_repo")

import numpy as np
import ml_dtypes
from numpy.lib.stride_tricks import as_strided

import concourse.bass as bass
import concourse.mybir as mybir
import concourse.tile as tile
from concourse import bacc
from concourse.bass_utils import run_bass_kernel_spmd

B, C, W, H = 4, 128, 128, 128
DW = 21          # displacement window (per axis)
PAD = 10
N_CORES = 8
PI, PJ = 16, 8           # patch shape (pixel rows x pixel cols)
IB, JB = 64 // PI, H // PJ          # 4 x 16 patch grid per core
WR, WQ = PI + DW - 1, PJ + DW - 1   # window 36 x 28
NPP = WR * WQ            # 1008 window positions per patch
HNP = NPP // 2           # 504 = one PSUM bank worth
TRIM = DW * WQ           # 588 values kept per pixel (rows pi..pi+21)
F = JB * NPP             # 16128 row-buffer free size
HALO_ROWS = 64 + 2 * PAD     # 84
PADDED_COLS = H + 2 * PAD    # 148

BF16 = ml_dtypes.bfloat16

_CACHE = {}


def _build_program():
    nc = bacc.Bacc("TRN2", target_bir_lowering=False, debug=False,
                   num_devices=N_CORES)
    x1h = nc.dram_tensor("x1h", [C, HALO_ROWS, PADDED_COLS],
                         mybir.dt.bfloat16, kind="ExternalInput")
    x2s = nc.dram_tensor("x2s", [C, IB, JB, 128], mybir.dt.bfloat16,
                         kind="ExternalInput")
    outp = nc.dram_tensor("outp", [IB, PI, PJ, JB, TRIM], mybir.dt.float16,
                          kind="ExternalOutput")

    with tile.TileContext(nc) as tc:
        with (
            tc.tile_pool(name="singles", bufs=1) as singles,
            tc.tile_pool(name="rowbuf", bufs=2) as rowbuf,
            tc.tile_pool(name="psum", bufs=4, space="PSUM") as psum,
        ):
            x1_sb = singles.tile([C, HALO_ROWS, PADDED_COLS],
                                 mybir.dt.bfloat16)
            x2_sb = singles.tile([C, IB, JB, 128], mybir.dt.bfloat16)
            # chunked input loads so the first patches start early
            nc.sync.dma_start(out=x1_sb[:, 0:18, :], in_=x1h[:, 0:18, :])
            nc.sync.dma_start(out=x2_sb[:, 0:1], in_=x2s[:, 0:1])
            nc.sync.dma_start(out=x1_sb[:, 18:36, :], in_=x1h[:, 18:36, :])
            nc.sync.dma_start(out=x1_sb[:, 36:52, :], in_=x1h[:, 36:52, :])
            nc.sync.dma_start(out=x2_sb[:, 1:2], in_=x2s[:, 1:2])
            nc.sync.dma_start(out=x1_sb[:, 52:68, :], in_=x1h[:, 52:68, :])
            nc.sync.dma_start(out=x2_sb[:, 2:3], in_=x2s[:, 2:3])
            nc.sync.dma_start(out=x1_sb[:, 68:HALO_ROWS, :],
                              in_=x1h[:, 68:HALO_ROWS, :])
            nc.sync.dma_start(out=x2_sb[:, 3:4], in_=x2s[:, 3:4])

            for ib in range(IB):
                rb = rowbuf.tile([128, F], mybir.dt.float16)
                for jb in range(JB):
                    lhsT = x2_sb[:, ib, jb, :]
                    ps0 = psum.tile([128, HNP], mybir.dt.float32, name="ps0")
                    ps1 = psum.tile([128, HNP], mybir.dt.float32, name="ps1")
                    r0, c0 = ib * PI, jb * PJ
                    for r in range(WR // 2):
                        nc.tensor.matmul(
                            ps0[:, r * WQ:(r + 1) * WQ], lhsT=lhsT,
                            rhs=x1_sb[:, r0 + r, c0:c0 + WQ],
                            start=True, stop=True)
                    for r in range(WR // 2):
                        nc.tensor.matmul(
                            ps1[:, r * WQ:(r + 1) * WQ], lhsT=lhsT,
                            rhs=x1_sb[:, r0 + WR // 2 + r, c0:c0 + WQ],
                            start=True, stop=True)
                    base = jb * NPP
                    nc.vector.tensor_copy(rb[:, base:base + HNP], ps0)
                    nc.scalar.copy(out=rb[:, base + HNP:base + NPP], in_=ps1)
                # trimmed output: group g = pixel row pi, partitions
                # [8g, 8g+8), free offset g*28, runs of 588
                for g in range(PI):
                    src = bass.AP(rb.tensor, PJ * g * F + g * WQ,
                                  [[F, PJ], [NPP, JB], [1, TRIM]])
                    if ib == IB - 1:
                        # last row drains with nothing to overlap: spread
                        # issue across SWDGE (Pool) and HWDGE (SP) evenly
                        eng = nc.gpsimd if g % 8 in (1, 3, 5) else nc.sync
                    else:
                        eng = nc.gpsimd if g % 4 == 3 else nc.sync
                    eng.dma_start(out=outp[ib, g], in_=src)

    nc.finalize()
    return nc


def _shard_inputs(x1, x2):
    x1 = np.asarray(x1, dtype=np.float32)
    x2 = np.asarray(x2, dtype=np.float32)
    x1b = x1.astype(BF16)
    x2b = x2.astype(BF16)
    in_maps = []
    for k in range(N_CORES):
        b, half = divmod(k, 2)
        i0 = 64 * half
        x2sh = np.ascontiguousarray(
            x2b[b][:, i0:i0 + 64, :]
            .reshape(C, IB, PI, JB, PJ)
            .transpose(0, 1, 3, 2, 4)
            .reshape(C, IB, JB, 128)
        )
        x1sh = np.zeros((C, HALO_ROWS, PADDED_COLS), BF16)
        rlo, rhi = i0 - PAD, i0 + 64 + PAD
        slo, shi = max(rlo, 0), min(rhi, W)
        x1sh[:, slo - rlo:shi - rlo, PAD:PAD + H] = x1b[b][:, slo:shi, :]
        in_maps.append({"x1h": x1sh, "x2s": x2sh})
    return in_maps


def _gather(results):
    out = np.empty((B, DW * DW, W, H), np.float32)
    for k in range(N_CORES):
        b, half = divmod(k, 2)
        i0 = 64 * half
        O = np.ascontiguousarray(results[k]["outp"])  # [4, 16, 8, 16, 588]
        O6 = O.reshape(IB, PI, PJ, JB, DW, WQ)
        s = O6.strides
        # pixel (pi, pj) row di holds window cols [0, 28); its dj block
        # starts at col pj: bump the pj stride by the dj stride
        V = as_strided(O6, shape=(IB, PI, PJ, JB, DW, DW),
                       strides=(s[0], s[1], s[2] + s[5], s[3], s[4], s[5]))
        arr = V.transpose(4, 5, 0, 1, 3, 2).reshape(DW * DW, 64, H)
        out[b, :, i0:i0 + 64, :] = arr
    return out


def kernel(x1, x2):
    if "nc" not in _CACHE:
        _CACHE["nc"] = _build_program()
    nc = _CACHE["nc"]
    in_maps = _shard_inputs(x1, x2)
    res = run_bass_kernel_spmd(nc, in_maps, list(range(N_CORES)))
    return _gather(res.results)
